# revision 45
# baseline (speedup 1.0000x reference)
"""LGCN encoder kernel for 8 Trainium2 NeuronCores.

Computes out = 0.5*(ego + V @ (filt[:,None] * (V^T @ ego))) with
ego = concat(user_emb, item_emb), row-sharded over N across 8 cores.
The [F, D] projection partial is AllReduced across cores.

Default variant ("c"): V streams from HBM exactly once, in fp16, in
row-major [n, f] layout.  Pass 1 consumes each [128, 512] tile as the
matmul's stationary operand (moving operand is the 64-wide ego tile),
and the PE additionally transposes each tile (identity matmul) into a
resident SBUF copy with f on partitions that pass 2 reads back —
nothing reads V from HBM twice.  The [F, D] proj partial is AllReduced
in fp16; the last 16 tiles are re-read and transposed during the
collective's latency window so DMA/PE/DVE stay busy (and the PE HAM
clock stays warm) until pass 2 can start.  All streams are fp16 (PSUM
accumulation stays fp32); end-to-end error ~6e-4 vs the 2e-2 gate.
ego arrives host-side pre-scaled by 0.5 and tile-shuffled so one
tensor serves pass 1 and the epilogue; outputs leave in the same
shuffled [p, t, d] layout and the host unshuffles.

Variant "a" (kept as fallback): V is passed in both layouts
(host-transposed) and streamed twice, with the AllReduce hidden under
the second stream.  Measured ~118us vs ~78us for variant "c" on HW
(reps-contrast wall-clock method; no NTFF profiling under this axon
build).
"""

import sys

if "/opt/trn_rl_repo" not in sys.path:
    sys.path.insert(0, "/opt/trn_rl_repo")

import ml_dtypes
import numpy as np

from concourse import bacc, bass, mybir, tile
from concourse.bass_utils import run_bass_kernel_spmd

N_CORES = 8
USER_NUM = 50000
ITEM_NUM = 50000
N_FULL = USER_NUM + ITEM_NUM          # 100000
F = 512
D = 64
P = 128                               # partitions / n-tile rows
ROWS = 12800                          # rows per core, 100 tiles of 128
NPAD = ROWS * N_CORES                 # 102400
N_TILES = ROWS // P                   # 100
BLK = 512                             # pass-2 n-block (free dim)
N_BLKS = ROWS // BLK                  # 25
FC = F // P                           # 4 f-chunks of 128

F32 = mybir.dt.float32
F16 = mybir.dt.float16
MM_DT_DEFAULT = F16


def _np_of(dt):
    return {
        mybir.dt.float16: np.float16,
        mybir.dt.bfloat16: ml_dtypes.bfloat16,
        mybir.dt.float32: np.float32,
        mybir.dt.float32r: np.float32,
    }[dt]


def _build(mm_dt=None, single_core=False, reps=1):
    MM_DT = mm_dt if mm_dt is not None else MM_DT_DEFAULT
    nc = bacc.Bacc(
        "TRN2",
        target_bir_lowering=False,
        debug=False,
        num_devices=1 if single_core else N_CORES,
    )
    v_rows = nc.dram_tensor("v_rows", [ROWS, F], MM_DT, kind="ExternalInput").ap()
    v_cols = nc.dram_tensor("v_cols", [F, ROWS], MM_DT, kind="ExternalInput").ap()
    # ego arrives host-shuffled to [p, t, d] (t = n-tile index, n = t*128+p)
    # so the load is one fully-linear DMA. egoh = 0.5*ego^T feeds the
    # transposed epilogue. out is stored transposed [d, n]; the host
    # transposes it back.
    ego = nc.dram_tensor(
        "ego", [P, N_TILES * D], MM_DT, kind="ExternalInput"
    ).ap()
    egoh = nc.dram_tensor(
        "egoh", [D, ROWS], F16, kind="ExternalInput"
    ).ap()
    filt = nc.dram_tensor("filt", [F], F32, kind="ExternalInput").ap()
    ident = nc.dram_tensor("ident", [D, D], F32, kind="ExternalInput").ap()
    out = nc.dram_tensor("out", [D, ROWS], F16, kind="ExternalOutput").ap()

    # group sizes taper at the end so the last loads (which gate the
    # AllReduce chain / the final drain) complete quickly
    V_GROUPS = [13] * 7 + [4, 3, 2]       # n-tiles per v-row DMA
    # vT groups start small so the tiny AllReduce bounce DMAs are not
    # stuck behind multi-MB transfers, and end small to shrink the drain
    VT_GROUPS = [1, 2, 4, 6, 6, 3, 2, 1]  # pass-2 blocks per vT DMA

    with tile.TileContext(nc) as tc:
        with (
            tc.tile_pool(name="const", bufs=1) as const_pool,
            tc.tile_pool(name="stream", bufs=3) as stream_pool,
            tc.tile_pool(name="small", bufs=1) as small_pool,
            tc.tile_pool(name="outp", bufs=4) as out_pool,
            tc.tile_pool(name="ps_proj", bufs=1, space="PSUM") as ps_proj,
            tc.tile_pool(name="ps_ft", bufs=2, space="PSUM") as ps_ft,
            tc.tile_pool(name="ps_tr", bufs=4, space="PSUM") as ps_tr,
            tc.tile_pool(name="dram", bufs=2, space="DRAM") as dram_pool,
        ):
            for rep in range(reps):
                ident_sb = const_pool.tile([D, D], F32, tag="ident")
                nc.sync.dma_start(out=ident_sb[:], in_=ident[:])
                filt_sb = const_pool.tile([P, FC], F32, tag="filt")
                for c in range(FC):
                    nc.sync.dma_start(
                        out=filt_sb[:, c : c + 1], in_=filt[c * P : (c + 1) * P]
                    )
                # whole ego shard cached in SBUF (pass-1 lhsT tiles)
                ego_all = const_pool.tile([P, N_TILES, D], MM_DT, tag="ego_all")
                nc.sync.dma_start(
                    out=ego_all[:], in_=ego.rearrange("p (t d) -> p t d", d=D)
                )
                # 0.5*ego^T for the transposed epilogue
                egoh_sb = const_pool.tile([D, ROWS], F16, tag="egoh")
                nc.sync.dma_start(out=egoh_sb[:], in_=egoh[:])

                # ---- pass 1: projT[d, f] += sum_n ego[n, d] * v[n, f] ----
                projT_ps = ps_proj.tile([D, F], F32, tag="projT")
                t0 = 0
                for vg in V_GROUPS:
                    v_g = stream_pool.tile([P, vg, F], MM_DT, tag="strm")
                    nc.sync.dma_start(
                        out=v_g[:],
                        in_=v_rows[t0 * P : (t0 + vg) * P, :].rearrange(
                            "(j p) f -> p j f", p=P
                        ),
                    )
                    for j in range(vg):
                        t = t0 + j
                        nc.tensor.matmul(
                            projT_ps[:],
                            lhsT=ego_all[:, t, :],
                            rhs=v_g[:, j, :],
                            start=(t == 0),
                            stop=(t == N_TILES - 1),
                        )
                    t0 += vg

                # ---- AllReduce the [D, F] partial over all 8 cores ----
                projT_sb = small_pool.tile([D, F], F32, tag="projT_sb")
                nc.vector.tensor_copy(projT_sb[:], projT_ps[:])
                ar_in = dram_pool.tile([D, F], F32, tag="ar_in")
                ar_out = dram_pool.tile([D, F], F32, tag="ar_out")
                nc.scalar.dma_start(out=ar_in[:], in_=projT_sb[:])
                if single_core:
                    nc.scalar.dma_start(out=ar_out[:], in_=ar_in[:])
                else:
                    nc.gpsimd.collective_compute(
                        "AllReduce",
                        mybir.AluOpType.add,
                        replica_groups=[list(range(N_CORES))],
                        ins=[ar_in.opt()],
                        outs=[ar_out.opt()],
                    )
                projT_all = small_pool.tile([D, F], F32, tag="projT_all")
                nc.scalar.dma_start(out=projT_all[:], in_=ar_out[:])

                # ---- M[f, d] = 0.5 * filt[f] * proj[f, d], in 4 chunks ----
                m_chunks = []
                for c in range(FC):
                    tr_ps = ps_tr.tile([P, D], F32, tag="tr")
                    nc.tensor.transpose(
                        tr_ps[:], projT_all[:, c * P : (c + 1) * P], ident_sb[:]
                    )
                    m_sb = small_pool.tile([P, D], MM_DT, tag=f"m{c}")
                    nc.vector.tensor_scalar(
                        out=m_sb[:],
                        in0=tr_ps[:],
                        scalar1=filt_sb[:, c : c + 1],
                        scalar2=0.5,
                        op0=mybir.AluOpType.mult,
                        op1=mybir.AluOpType.mult,
                    )
                    m_chunks.append(m_sb)

                # ---- pass 2: filteredT[d, n] = sum_f M[f, d] * vT[f, n] ----
                b0 = 0
                for tg in VT_GROUPS:
                    vt_g = stream_pool.tile([P, FC, tg * BLK], MM_DT, tag="strm")
                    nc.sync.dma_start(
                        out=vt_g[:],
                        in_=v_cols[:, b0 * BLK : (b0 + tg) * BLK].rearrange(
                            "(c p) n -> p c n", p=P
                        ),
                    )
                    for bb in range(tg):
                        b = b0 + bb
                        ftT_ps = ps_ft.tile([D, BLK], F32, tag="ftT")
                        for c in range(FC):
                            nc.tensor.matmul(
                                ftT_ps[:],
                                lhsT=m_chunks[c][:],
                                rhs=vt_g[:, c, bb * BLK : (bb + 1) * BLK],
                                start=(c == 0),
                                stop=(c == FC - 1),
                            )
                        # epilogue: out^T = filtered^T + 0.5*ego^T, straight
                        # from PSUM, stored transposed
                        out_blk = out_pool.tile([D, BLK], F16, tag="o")
                        nc.vector.tensor_add(
                            out_blk[:],
                            ftT_ps[:],
                            egoh_sb[:, b * BLK : (b + 1) * BLK],
                        )
                        nc.scalar.dma_start(
                            out=out[:, b * BLK : (b + 1) * BLK], in_=out_blk[:]
                        )
                    b0 += tg

    nc.compile()
    return nc


# ---------------------------------------------------------------------------
# Variant C: stream V once ([n, f] layout), build the [f, n] copy on-device
# with PE transposes into a resident SBUF buffer, except for a small tail
# streamed as pre-transposed v_cols so DMA work covers the AllReduce window.
# Both matmul passes use the V tile as the stationary operand (64-wide
# moving operand), and proj comes out in [f, d] directly so no M transpose
# is needed.  ego arrives pre-scaled by 0.5 and serves pass 1, and the
# epilogue; output leaves in the same shuffled [p, t, d] layout.
# ---------------------------------------------------------------------------

# v group sizes (n-tiles per DMA).  Each partition receives a CONTIGUOUS
# run of vg rows ("(p j) f -> p j f"), so every group is one descriptor
# per partition at full line rate; ego/out use the same row permutation
# (host-side).  The last groups are 4 tiles so the re-read groups (also
# 4 tiles) map rows identically.
VG_LAYOUT = [20, 20, 20, 20, 8, 4, 4, 4]
T_TAIL = 8                            # tiles re-read + transposed during AR
T_TR = N_TILES - T_TAIL               # tiles transposed during the stream
RR_G = 4                              # re-read group size (must match tail
                                      # groups of VG_LAYOUT)


def _row_of():
    """row_of[p, t] = v-shard row held by partition p for logical tile t."""
    row_of = np.empty((P, N_TILES), np.int64)
    t0 = 0
    for vg in VG_LAYOUT:
        for j in range(vg):
            row_of[:, t0 + j] = t0 * P + np.arange(P) * vg + j
        t0 += vg
    return row_of



def _build_c(reps=1):
    t_tr = T_TR
    nc = bacc.Bacc(
        "TRN2",
        target_bir_lowering=False,
        debug=False,
        num_devices=N_CORES,
    )
    v_rows = nc.dram_tensor("v_rows", [ROWS, F], F16, kind="ExternalInput").ap()
    ego05 = nc.dram_tensor(
        "ego05", [P, N_TILES * D], F16, kind="ExternalInput"
    ).ap()
    # filt arrives pre-shaped [P, FC] (partition-major chunks): one linear DMA
    filt = nc.dram_tensor("filt", [P, FC], F32, kind="ExternalInput").ap()
    identc = nc.dram_tensor("identc", [P, P], F16, kind="ExternalInput").ap()
    out = nc.dram_tensor("out", [P, N_TILES * D], F16, kind="ExternalOutput").ap()
    out_r = out.rearrange("p (t d) -> p t d", d=D)

    with tile.TileContext(nc) as tc:
        with (
            tc.tile_pool(name="const", bufs=1) as const_pool,
            tc.tile_pool(name="stream", bufs=3) as stream_pool,
            tc.tile_pool(name="small", bufs=1) as small_pool,
            tc.tile_pool(name="outp", bufs=3) as out_pool,
            tc.tile_pool(name="ps_proj", bufs=1, space="PSUM") as ps_proj,
            tc.tile_pool(name="ps_vtr", bufs=3, space="PSUM") as ps_vtr,
            tc.tile_pool(name="ps_ft", bufs=3, space="PSUM") as ps_ft,
            tc.tile_pool(name="dram", bufs=2, space="DRAM") as dram_pool,
        ):
            for rep in range(reps):
                # const loads go on the scalar queue so the v stream owns
                # the sync queue from t=0
                identc_sb = const_pool.tile([P, P], F16, tag="identc")
                nc.scalar.dma_start(out=identc_sb[:], in_=identc[:])
                filt_sb = const_pool.tile([P, FC], F32, tag="filt")
                nc.scalar.dma_start(out=filt_sb[:], in_=filt[:])
                ego_all = const_pool.tile([P, N_TILES, D], F16, tag="ego05")
                nc.scalar.dma_start(
                    out=ego_all[:], in_=ego05.rearrange("p (t d) -> p t d", d=D)
                )
                # on-device-transposed copy of v (f on partitions); the last
                # t_tail tiles are re-read and transposed DURING the AllReduce
                # so PE/DVE stay busy (and warm) through the collective
                vt_sb = const_pool.tile([P, FC, N_TILES * P], F16, tag="vt_sb")

                cp_state = {"i": 0}

                def transpose_tile(v_g, j, t):
                    tr = ps_vtr.tile([P, FC, P], F16, tag="vtr")
                    for c in range(FC):
                        nc.tensor.matmul(
                            tr[:, c, :],
                            lhsT=v_g[:, j, c * P : (c + 1) * P],
                            rhs=identc_sb[:],
                            is_transpose=True,
                            start=(c == 0),
                            stop=(c == FC - 1),
                        )
                    # gpsimd cannot touch PSUM on HW: alternate the DVE copy
                    # with an Activation-engine Copy
                    if cp_state["i"] % 2 == 0:
                        nc.vector.tensor_copy(
                            vt_sb[:, :, t * P : (t + 1) * P], tr[:]
                        )
                    else:
                        nc.scalar.activation(
                            vt_sb[:, :, t * P : (t + 1) * P],
                            tr[:],
                            mybir.ActivationFunctionType.Copy,
                        )
                    cp_state["i"] += 1

                # ---- pass 1 (+ tile transposes into vt_sb) ----
                proj_ps = ps_proj.tile([P, FC * D], F32, tag="proj")
                t0 = 0
                for vg in VG_LAYOUT:
                    v_g = stream_pool.tile([P, vg, F], F16, tag="strm")
                    # linear load: partition p takes vg consecutive rows —
                    # one descriptor per partition at full line rate
                    nc.sync.dma_start(
                        out=v_g[:],
                        in_=v_rows[t0 * P : (t0 + vg) * P, :].rearrange(
                            "(p j) f -> p j f", j=vg
                        ),
                    )
                    for j in range(vg):
                        t = t0 + j
                        for c in range(FC):
                            # one zero region: single start/stop pair
                            nc.tensor.matmul(
                                proj_ps[:, c * D : (c + 1) * D],
                                lhsT=v_g[:, j, c * P : (c + 1) * P],
                                rhs=ego_all[:, t, :],
                                start=(t == 0 and c == 0),
                                stop=(t == N_TILES - 1 and c == FC - 1),
                            )
                        if t < t_tr:
                            transpose_tile(v_g, j, t)
                    t0 += vg

                # ---- AllReduce proj over the 8 cores (elementwise, fp16) ----
                proj_sb = small_pool.tile([P, FC * D], F16, tag="proj_sb")
                nc.vector.tensor_copy(proj_sb[:], proj_ps[:])
                ar_in = dram_pool.tile([P, FC * D], F16, tag="ar_in")
                ar_out = dram_pool.tile([P, FC * D], F16, tag="ar_out")
                nc.scalar.dma_start(out=ar_in[:], in_=proj_sb[:])
                nc.gpsimd.collective_compute(
                    "AllReduce",
                    mybir.AluOpType.add,
                    replica_groups=[list(range(N_CORES))],
                    ins=[ar_in.opt()],
                    outs=[ar_out.opt()],
                )
                projall_sb = small_pool.tile([P, FC * D], F16, tag="projall")
                nc.scalar.dma_start(out=projall_sb[:], in_=ar_out[:])

                # ---- re-read + transpose the last tiles DURING the AR ----
                # (issued after the AR trigger; fills the collective's
                # latency window with useful PE/DVE work and keeps the
                # HAM clock warm for pass 2)
                t0 = t_tr
                while t0 < N_TILES:
                    vg = min(RR_G, N_TILES - t0)
                    v_g = stream_pool.tile([P, vg, F], F16, tag="strm")
                    # same linear layout as the main groups of size RR_G,
                    # so ego/out row permutation matches
                    nc.sync.dma_start(
                        out=v_g[:],
                        in_=v_rows[t0 * P : (t0 + vg) * P, :].rearrange(
                            "(p j) f -> p j f", j=vg
                        ),
                    )
                    for j in range(vg):
                        t = t0 + j
                        tr = ps_vtr.tile([P, FC, P], F16, tag="vtr")
                        for c in range(FC):
                            nc.tensor.matmul(
                                tr[:, c, :],
                                lhsT=v_g[:, j, c * P : (c + 1) * P],
                                rhs=identc_sb[:],
                                is_transpose=True,
                                start=(c == 0),
                                stop=(c == FC - 1),
                            )
                        # DVE only: the ACT queue must stay [ar_in, projall]
                        # so the post-AR chain is never stuck behind copies
                        nc.vector.tensor_copy(
                            vt_sb[:, :, t * P : (t + 1) * P], tr[:]
                        )
                    t0 += vg

                # ---- M[f, d] = filt[f] * proj[f, d] (0.5 folded into ego) ----
                m_all = small_pool.tile([P, FC, D], F16, tag="m_all")
                for c in range(FC):
                    nc.vector.tensor_scalar(
                        out=m_all[:, c, :],
                        in0=projall_sb[:, c * D : (c + 1) * D],
                        scalar1=filt_sb[:, c : c + 1],
                        scalar2=None,
                        op0=mybir.AluOpType.mult,
                    )

                # ---- pass 2: filtered[n, d] = sum_f vT[f, n] * M[f, d] ----
                # groups of 8 tiles fill one psum zero region exactly; out
                # stores ride the sync queue, idle once the stream is done
                t0g = 0
                while t0g < N_TILES:
                    # taper the last groups so the drain chain is short
                    gsz = min(8 if t0g < 96 else 2, N_TILES - t0g)
                    ft = ps_ft.tile([P, 8, D], F32, tag="ft")
                    for k in range(gsz):
                        t = t0g + k
                        src = vt_sb[:, :, t * P : (t + 1) * P]
                        for c in range(FC):
                            nc.tensor.matmul(
                                ft[:, k, :],
                                lhsT=src[:, c, :],
                                rhs=m_all[:, c, :],
                                start=(k == 0 and c == 0),
                                stop=(k == gsz - 1 and c == FC - 1),
                            )
                    ob = out_pool.tile([P, 8, D], F16, tag="o")
                    nc.vector.tensor_add(
                        ob[:, :gsz, :],
                        ft[:, :gsz, :],
                        ego_all[:, t0g : t0g + gsz, :],
                    )
                    nc.sync.dma_start(
                        out=out_r[:, t0g : t0g + gsz, :], in_=ob[:, :gsz, :]
                    )
                    t0g += gsz

    nc.compile()
    return nc


def _prep_in_maps_c(user_emb, item_emb, v, filt):
    ego = np.concatenate(
        [np.asarray(user_emb, np.float32), np.asarray(item_emb, np.float32)], axis=0
    )
    filt = np.asarray(filt, np.float32)
    ego_pad = np.zeros((NPAD, D), np.float32)
    ego_pad[:N_FULL] = ego
    v_pad = np.zeros((NPAD, F), np.float16)
    v_pad[:N_FULL] = np.asarray(v, np.float32)
    identc = np.eye(P, dtype=np.float16)
    # ego follows v's linear-load row permutation: partition p of logical
    # tile t holds shard row row_of[p, t]
    rows = _row_of().reshape(-1)  # [P * N_TILES]
    in_maps = []
    for c in range(N_CORES):
        sl = slice(c * ROWS, (c + 1) * ROWS)
        vr = np.ascontiguousarray(v_pad[sl])
        ego05 = np.ascontiguousarray(
            (0.5 * ego_pad[sl][rows]).astype(np.float16).reshape(P, N_TILES * D)
        )
        filt_pf = np.ascontiguousarray(filt.reshape(FC, P).T)
        in_maps.append(
            {
                "v_rows": vr,
                "ego05": ego05,
                "filt": filt_pf,
                "identc": identc,
            }
        )
    return in_maps


def _unshuffle_out_c(res):
    rows = _row_of().reshape(-1)  # [P * N_TILES]
    outs = []
    for c in range(N_CORES):
        o = np.asarray(res[c]["out"], np.float32).reshape(P * N_TILES, D)
        full = np.empty((ROWS, D), np.float32)
        full[rows] = o
        outs.append(full)
    return np.concatenate(outs, axis=0)[:N_FULL]


VARIANT = "c"

_NC = {}


def _get_nc(mm_dt=None, reps=1, variant=None):
    variant = variant or VARIANT
    key = (variant, mm_dt if mm_dt is not None else MM_DT_DEFAULT, reps)
    if key not in _NC:
        if variant == "c":
            _NC[key] = _build_c(reps=reps)
        else:
            _NC[key] = _build(key[1], reps=reps)
    return _NC[key]


def _prep_in_maps(user_emb, item_emb, v, filt, mm_dt=None):
    mm_np = _np_of(mm_dt if mm_dt is not None else MM_DT_DEFAULT)
    ego = np.concatenate(
        [np.asarray(user_emb, np.float32), np.asarray(item_emb, np.float32)], axis=0
    )
    v = np.asarray(v, np.float32)
    filt = np.asarray(filt, np.float32)
    ego_pad = np.zeros((NPAD, D), np.float32)
    ego_pad[:N_FULL] = ego
    v_pad = np.zeros((NPAD, F), mm_np)
    v_pad[:N_FULL] = v
    ident = np.eye(D, dtype=np.float32)
    in_maps = []
    for c in range(N_CORES):
        sl = slice(c * ROWS, (c + 1) * ROWS)
        vr = np.ascontiguousarray(v_pad[sl])
        ego_shuf = np.ascontiguousarray(
            ego_pad[sl]
            .reshape(N_TILES, P, D)
            .transpose(1, 0, 2)
            .reshape(P, N_TILES * D)
        ).astype(mm_np)
        egoh = np.ascontiguousarray(
            (0.5 * ego_pad[sl].T).astype(np.float16)
        )
        in_maps.append(
            {
                "v_rows": vr,
                "v_cols": np.ascontiguousarray(vr.T),
                "ego": ego_shuf,
                "egoh": egoh,
                "filt": filt,
                "ident": ident,
            }
        )
    return in_maps


def run(user_emb, item_emb, v, filt, trace=False, mm_dt=None, variant=None, **trace_kwargs):
    variant = variant or VARIANT
    nc = _get_nc(mm_dt, variant=variant)
    if variant == "c":
        in_maps = _prep_in_maps_c(user_emb, item_emb, v, filt)
    else:
        in_maps = _prep_in_maps(user_emb, item_emb, v, filt, mm_dt)
    res = run_bass_kernel_spmd(
        nc, in_maps, list(range(N_CORES)), trace=trace, **trace_kwargs
    )
    if variant == "c":
        out = _unshuffle_out_c(res.results)
    else:
        out = np.concatenate(
            [
                np.asarray(res.results[c]["out"], np.float32).T
                for c in range(N_CORES)
            ],
            axis=0,
        )[:N_FULL]
    return (out[:USER_NUM], out[USER_NUM:]), res


def kernel(user_emb, item_emb, v, filt, k=None, **_unused):
    (user_out, item_out), _ = run(user_emb, item_emb, v, filt)
    return (
        np.asarray(user_out, np.float32),
        np.asarray(item_out, np.float32),
    )
